# revision 39
# baseline (speedup 1.0000x reference)
"""Cost-volume concatenation kernel for Trainium2 (8 NeuronCores).

Reference (per batch b, disparity index d, i = d + MIN_DISP):
  out[b, d, h, w, 0:C]  = left[b, h, w, :]    if 0 <= w - i < W else 0
  out[b, d, h, w, C:2C] = right[b, h, w-i, :] if 0 <= w - i < W else 0

Sharding: disparity-parallel, interleaved -- core c builds disparities
{8j + c : j in 0..15} for the full [B, H, W] volume.  Interleaving
keeps the per-plane valid-span geometry nearly identical across cores
(spans differ by at most 7 columns), which the SPMD constraint needs.

Precision: the on-device datapath is int8, symmetric linear
quantization with one global scale s = max(|left|,|right|)_max / 127
computed on the host; the host dequantizes (one f32 multiply per
plane-block) on the way out.  Every output element is either a
quantized input value or an exact zero, so end-to-end error is the
quantization error <= s/2, i.e. rel-err <= 1/254 = 3.9e-3 against the
2e-2 budget.  int8 also minimizes device HBM traffic and, more
importantly, host<->device wire bytes (the end-to-end bottleneck).

Output layout (chosen so the HOST gather is one medium ufunc call per
plane with large contiguous runs -- profiling showed the previous
3-blocks-per-plane half-C layout cost ~6.5 s of strided numpy scatter
on the host, vs ~0.45 s for this one):
  * per plane j one packed block [H, B, span_j, 2C] int8, planes
    j = 0..15 packed back-to-back in one flat ExternalOutput.
  * the left|right channel halves are interleaved on DEVICE (vector/
    scalar engine copies into a [p, b, w, 2C] SBUF tile), so each
    block dequantizes into the final [b, d, h, w, 0:2C] view with a
    single np.multiply whose inner contiguous run is span*2C*4 bytes.
  * only the union-over-c valid span is stored per plane -- bytes on
    the wire are the metric that matters (axon tunnel ~30 MB/s), and
    the invalid remainder of the full volume is zeros the host gets
    for free from np.zeros.

SPMD trick: run_bass_kernel_spmd runs ONE program on all 8 cores, so
the per-core offset c cannot appear in any access pattern.  The
program is written for i0 = 8j - 112 and all c-dependence lives in
the data:
  * rightp input = right pre-shifted by +c columns, zero-padded to
    W+8 columns -- the program's static gather rightp[w - i0] then
    yields right[w - i] with the out-of-range mask applied by the
    padding.
  * left-half validity: inside the union span the mask can differ
    from 1 only on a 7-column edge (upper bound w < W + i0 + c for
    the i0 < 0 planes, lower bound w >= i0 + c for the i0 >= 0
    planes).  The interior is copied raw; the edge gets one tiny
    multiply with a 7-wide 0/1 mask built once from an iota and the
    per-core scalar input cvec (= c).
Each plane writes the union-over-c of valid w-spans; columns inside
the union but outside the core's true span receive exact zeros from
the padding / mask, so every byte of the output buffer is written.
"""

import os
import sys

sys.path.insert(0, "/opt/trn_rl_repo")

import numpy as np

B, H, W, C = 2, 96, 192, 16
D = 128
MIN_DISP = -112
N_CORES = 8
DPC = D // N_CORES         # 16 disparity planes per core
PAD = 8                    # rightp padded to W + PAD source columns
WP = W + PAD
ROWS = B * H

_CACHE = {}


def _plane_span(j):
    """Union-over-c valid w-span for plane j (program-static)."""
    i0 = 8 * j + MIN_DISP
    if i0 < 0:
        us, ue = 0, min(W + i0 + (N_CORES - 1), W)
    else:
        us, ue = i0, W
    return i0, us, ue


def _blocks():
    """Packed flat-output offsets: per plane one [H, B, nw, 2C] block."""
    off = {}
    o = 0
    for j in range(DPC):
        i0, us, ue = _plane_span(j)
        off[j] = o
        o += ROWS * (ue - us) * 2 * C
    return off, o


def _plane_order():
    """Zipper: widest, narrowest, 2nd-widest, ... balances DMA rings."""
    return sorted(range(DPC), key=lambda j: _plane_span(j)[1] - _plane_span(j)[2])


def _build_program():
    from concourse import bacc, mybir
    import concourse.tile as tile

    nc = bacc.Bacc(
        "TRN2", target_bir_lowering=False, debug=False, num_devices=N_CORES
    )
    i8 = mybir.dt.int8
    bf16 = mybir.dt.bfloat16
    f32 = mybir.dt.float32
    # Batch-packed inputs: partition q = image row h, free dims [b, cols].
    left = nc.dram_tensor("left", [H, B * W * C], i8, kind="ExternalInput")
    rightp = nc.dram_tensor("rightp", [H, B * WP * C], i8, kind="ExternalInput")
    cvec = nc.dram_tensor("cvec", [128, 1], f32, kind="ExternalInput")
    offs, total = _blocks()
    out = nc.dram_tensor("out", [total], i8, kind="ExternalOutput")

    def dst(j, b, nw):
        # Block memory order is [b, h, nw*2C] (b-major) so the host
        # dequant needs no [H,B] transpose.  One 2-D natural-order AP
        # per batch: a permuted [q r x] AP here routed the stores
        # through a GpSimd-assisted descriptor path (~71 us busy) --
        # natural-order APs keep them on the hardware DGE.
        o = offs[j] + b * H * nw * 2 * C
        return out.ap()[o : o + H * nw * 2 * C].rearrange("(q x) -> q x", q=H)

    E = 7  # edge width: union span minus guaranteed-valid interior

    with tile.TileContext(nc) as tc:
        with (
            tc.tile_pool(name="inputs", bufs=1) as ipool,
            tc.tile_pool(name="work", bufs=4) as wpool,
        ):
            lsb = ipool.tile([128, B * W * C], i8, tag="lsb")
            rsb = ipool.tile([128, B * WP * C], i8, tag="rsb")
            cv = ipool.tile([128, 1], f32, tag="cvec")
            tmpi = ipool.tile([128, B * E * C], bf16, tag="tmpi")
            emb = ipool.tile([128, B * E * C], bf16, tag="emb")
            em_lt = ipool.tile([128, B * E * C], i8, tag="em_lt")
            em_ge = ipool.tile([128, B * E * C], i8, tag="em_ge")

            # One load per input at the heads of the two (empty) HWDGE
            # store rings; cvec (512 B) leads the scalar ring.  iota
            # runs on GpSimd immediately (no deps).
            nc.scalar.dma_start(cv[:, :], cvec.ap())
            nc.sync.dma_start(rsb[0:96, :], rightp.ap())
            nc.scalar.dma_start(lsb[0:96, :], left.ap())
            nc.gpsimd.iota(
                tmpi[:, :], [[0, B], [1, E], [0, C]], channel_multiplier=0,
                allow_small_or_imprecise_dtypes=True,
            )

            # Edge masks over dw = 0..6 (batch- and channel-expanded),
            # exact integer compares in bf16, cast to int8 once:
            # em_lt[dw] = dw < c  (right edge of i0 < 0 planes),
            # em_ge[dw] = dw >= c (left edge of i0 >= 0 planes).
            nc.vector.tensor_single_scalar(
                emb[:, :], tmpi[:, :], cv[:, 0:1], mybir.AluOpType.is_lt
            )
            nc.scalar.copy(em_lt[:, :], emb[:, :])
            nc.vector.tensor_single_scalar(
                emb[:, :], tmpi[:, :], cv[:, 0:1], mybir.AluOpType.is_ge
            )
            nc.scalar.copy(em_ge[:, :], emb[:, :])

            lv = lsb[0:96, :].rearrange("p (r w y) -> p r w y", r=B, w=W)
            rv = rsb[0:96, :].rearrange("p (r w y) -> p r w y", r=B, w=WP)
            elv = em_lt[0:96, :].rearrange("p (r w y) -> p r w y", r=B, w=E)
            egv = em_ge[0:96, :].rearrange("p (r w y) -> p r w y", r=B, w=E)

            for k, j in enumerate(_plane_order()):
                i0, us, ue = _plane_span(j)
                nw = ue - us
                x0 = us - i0      # source column offset into rightp
                ring = (nc.sync, nc.scalar)[k % 2]
                # Edge start and always-valid interior [n0, n1).
                if i0 < 0:
                    e0, n0, n1, em = ue - E, us, ue - E, elv
                else:
                    e0, n0, n1, em = us, us + E, ue, egv

                cat = wpool.tile([128, B * W * 2 * C], i8, tag="cat")
                cw = cat[0:96, :].rearrange("p (r w y) -> p r w y", r=B, w=W)

                # Left interior (validity provably 1 for every core):
                # raw copy on the vector engine.
                nc.vector.tensor_copy(
                    cw[:, :, n0 - us : n1 - us, 0:C], lv[:, :, n0:n1, :]
                )
                # Left edge (<= 7 columns where the mask may be 0):
                # one tiny multiply.
                nc.vector.tensor_mul(
                    cw[:, :, e0 - us : e0 - us + E, 0:C],
                    lv[:, :, e0 : e0 + E, :],
                    em[:, :, :, :],
                )
                # Shifted right window into the odd channel half
                # (scalar engine; the disparity shift is just a byte
                # offset in the source AP).
                nc.scalar.copy(
                    cw[:, :, 0:nw, C : 2 * C], rv[:, :, x0 : x0 + nw, :]
                )

                # Two DMAs per plane (one per batch), both sides 2-D
                # natural-order with a contiguous nw*2C inner run; the
                # two rings each carry one batch of every plane.
                ring.dma_start(
                    dst(j, 0, nw), cat[0:96, 0 : nw * 2 * C]
                )
                ring2 = (nc.scalar, nc.sync)[k % 2]
                ring2.dma_start(
                    dst(j, 1, nw),
                    cat[0:96, W * 2 * C : W * 2 * C + nw * 2 * C],
                )

    nc.compile()
    return nc


def _get_program():
    if "nc" not in _CACHE:
        _CACHE["nc"] = _build_program()
    return _CACHE["nc"]


def _get_runner():
    """SPMD executor.

    On a native TRN host (no axon), defer to run_bass_kernel_spmd --
    the NrtSession path it picks there has local DMA and needs no help.

    Under axon (remote-accelerator PJRT proxy, ~30 MB/s tunnel), use
    the multi-core body of bass2jax.run_bass_via_pjrt -- the exact path
    run_bass_kernel_spmd takes under axon -- hoisted out of the
    per-call path so the jitted shard_map is built once, and with the
    donated ExternalOutput buffers created ON DEVICE (a jnp.zeros jit)
    instead of uploading ~114 MB of host zeros through the tunnel on
    every call.  This program writes every output byte, so the donated
    buffer's contents are never observed anyway.
    """
    if "runner" in _CACHE:
        return _CACHE["runner"]

    from concourse._compat import axon_active

    if not axon_active():
        from concourse.bass_utils import run_bass_kernel_spmd

        nc = _get_program()

        def run_native(in_maps):
            res = run_bass_kernel_spmd(
                nc, in_maps, core_ids=list(range(N_CORES))
            )
            return [res.results[c]["out"] for c in range(N_CORES)]

        _CACHE["runner"] = run_native
        return run_native

    import jax
    import jax.numpy as jnp
    from jax.sharding import Mesh, NamedSharding, PartitionSpec
    from jax.experimental.shard_map import shard_map
    from concourse import bass2jax, mybir

    nc = _get_program()
    bass2jax.install_neuronx_cc_hook()

    partition_name = (
        nc.partition_id_tensor.name if nc.partition_id_tensor else None
    )
    in_names, out_names, out_avals = [], [], []
    for alloc in nc.m.functions[0].allocations:
        if not isinstance(alloc, mybir.MemoryLocationSet):
            continue
        name = alloc.memorylocations[0].name
        if alloc.kind == "ExternalInput":
            if name != partition_name:
                in_names.append(name)
        elif alloc.kind == "ExternalOutput":
            out_names.append(name)
            out_avals.append(
                jax.core.ShapedArray(
                    tuple(alloc.tensor_shape), mybir.dt.np(alloc.dtype)
                )
            )
    n_params = len(in_names)
    all_names = list(in_names) + out_names
    if partition_name is not None:
        all_names.append(partition_name)

    def _body(*args):
        operands = list(args)
        if partition_name is not None:
            operands.append(bass2jax.partition_id_tensor())
        outs = bass2jax._bass_exec_p.bind(
            *operands,
            out_avals=tuple(out_avals),
            in_names=tuple(all_names),
            out_names=tuple(out_names),
            lowering_input_output_aliases=(),
            sim_require_finite=True,
            sim_require_nnan=True,
            nc=nc,
        )
        return tuple(outs)

    devices = jax.devices()[:N_CORES]
    mesh = Mesh(np.asarray(devices), ("core",))
    donate = tuple(range(n_params, n_params + len(out_names)))
    sharded = jax.jit(
        shard_map(
            _body,
            mesh=mesh,
            in_specs=(PartitionSpec("core"),) * (n_params + len(out_names)),
            out_specs=(PartitionSpec("core"),) * len(out_names),
            check_rep=False,
        ),
        donate_argnums=donate,
        keep_unused=True,
    )
    zshape = (N_CORES * out_avals[0].shape[0], *out_avals[0].shape[1:])
    zeros_fn = jax.jit(
        lambda: jnp.zeros(zshape, jnp.int8),
        out_shardings=NamedSharding(mesh, PartitionSpec("core")),
    )

    def run(in_maps):
        """Returns the sharded jax output array; the caller fetches
        its shards (kernel() overlaps the fetch with dequantization)."""
        concat_in = [
            np.concatenate([m[name] for m in in_maps], axis=0)
            for name in in_names
        ]
        return sharded(*concat_in, zeros_fn())[0]

    _CACHE["sharded"] = sharded
    _CACHE["zeros_fn"] = zeros_fn
    _CACHE["in_names"] = in_names
    _CACHE["runner"] = run
    return run


def kernel(left, right):
    left = np.ascontiguousarray(left, dtype=np.float32)
    right = np.ascontiguousarray(right, dtype=np.float32)
    scale = max(np.abs(left).max(), np.abs(right).max()) / 127.0
    scale = float(scale) if scale > 0 else 1.0
    left_q = np.clip(np.rint(left / scale), -127, 127).astype(np.int8)
    right_q = np.clip(np.rint(right / scale), -127, 127).astype(np.int8)
    # Batch-packed: [h, b, cols].
    left_t = np.ascontiguousarray(
        left_q.reshape(B, H, W * C).transpose(1, 0, 2)
    ).reshape(H, B * W * C)

    in_maps = []
    for c in range(N_CORES):
        rp = np.zeros((B, H, WP, C), dtype=np.int8)
        rp[:, :, c : c + W] = right_q
        rp_t = np.ascontiguousarray(
            rp.reshape(B, H, WP * C).transpose(1, 0, 2)
        ).reshape(H, B * WP * C)
        cvv = np.full((128, 1), float(c), dtype=np.float32)
        in_maps.append({"left": left_t, "rightp": rp_t, "cvec": cvv})

    run = _get_runner()
    prof_dir = os.environ.get("BASS_NTFF_DIR")
    ctx = None
    if prof_dir:
        try:  # optional NTFF capture; absent outside the axon dev env
            from trn_agent_boot.trn_boot import _ntff_profile_via_ctypes

            ctx = _ntff_profile_via_ctypes("/opt/axon/libaxon_pjrt.so")(
                prof_dir, [0]
            )
        except Exception:
            ctx = None
    if ctx is None:
        import contextlib

        ctx = contextlib.nullcontext()

    # Gather: one dequantizing multiply per plane-block into the final
    # [B, D, H, W, 2C] array (d = 8j + c).  Blocks arrive [b, h, nw, 2C]
    # so no transpose is needed; invalid columns outside the union
    # spans stay zero from calloc.
    offs, total = _blocks()
    s32 = np.float32(scale)
    full = np.zeros((B, DPC, N_CORES, H, W, 2 * C), dtype=np.float32)

    def unpack_core(c, flat):
        for j in range(DPC):
            i0, us, ue = _plane_span(j)
            nw = ue - us
            o = offs[j]
            blk = flat[o : o + ROWS * nw * 2 * C].reshape(B, H, nw, 2 * C)
            np.multiply(blk, s32, out=full[:, j, c, :, us:ue, :])

    with ctx:
        results = run(in_maps)
        if hasattr(results, "addressable_shards"):
            # Axon path: fetch the 8 shards on concurrent threads (the
            # D2H transport runs in C with the GIL released) and
            # dequantize each core's blocks as it lands, overlapping
            # host math with the remaining wire time.
            import queue
            import threading

            q = queue.Queue()

            def fetch(sh):
                try:
                    q.put((sh.index[0].start // total, np.asarray(sh.data)))
                except BaseException as e:
                    q.put(e)

            threads = [
                threading.Thread(target=fetch, args=(sh,))
                for sh in results.addressable_shards
            ]
            for t in threads:
                t.start()
            for _ in range(N_CORES):
                item = q.get()
                if isinstance(item, BaseException):
                    for t in threads:
                        t.join()
                    raise item
                unpack_core(*item)
            for t in threads:
                t.join()
        else:
            # Native path: run_bass_kernel_spmd already returned
            # host-resident per-core arrays.
            for c in range(N_CORES):
                unpack_core(c, results[c])
    return full.reshape(B, D, H, W, 2 * C)


# revision 40
# speedup vs baseline: 1.0015x; 1.0015x over previous
"""Cost-volume concatenation kernel for Trainium2 (8 NeuronCores).

Reference (per batch b, disparity index d, i = d + MIN_DISP):
  out[b, d, h, w, 0:C]  = left[b, h, w, :]    if 0 <= w - i < W else 0
  out[b, d, h, w, C:2C] = right[b, h, w-i, :] if 0 <= w - i < W else 0

Sharding: disparity-parallel, interleaved -- core c builds disparities
{8j + c : j in 0..15} for the full [B, H, W] volume.  Interleaving
keeps the per-plane valid-span geometry nearly identical across cores
(spans differ by at most 7 columns), which the SPMD constraint needs.

Precision: the on-device datapath is int8, symmetric linear
quantization with one global scale s = max(|left|,|right|)_max / 127
computed on the host; the host dequantizes (one f32 multiply per
plane-block) on the way out.  Every output element is either a
quantized input value or an exact zero, so end-to-end error is the
quantization error <= s/2, i.e. rel-err <= 1/254 = 3.9e-3 against the
2e-2 budget.  int8 also minimizes device HBM traffic and, more
importantly, host<->device wire bytes (the end-to-end bottleneck).

Output layout (chosen so the HOST gather is one medium ufunc call per
plane with large contiguous runs -- profiling showed the previous
3-blocks-per-plane half-C layout cost ~6.5 s of strided numpy scatter
on the host, vs ~0.45 s for this one):
  * per plane j one packed block [H, B, span_j, 2C] int8, planes
    j = 0..15 packed back-to-back in one flat ExternalOutput.
  * the left|right channel halves are interleaved on DEVICE (vector/
    scalar engine copies into a [p, b, w, 2C] SBUF tile), so each
    block dequantizes into the final [b, d, h, w, 0:2C] view with a
    single np.multiply whose inner contiguous run is span*2C*4 bytes.
  * only the union-over-c valid span is stored per plane -- bytes on
    the wire are the metric that matters (axon tunnel ~30 MB/s), and
    the invalid remainder of the full volume is zeros the host gets
    for free from np.zeros.

SPMD trick: run_bass_kernel_spmd runs ONE program on all 8 cores, so
the per-core offset c cannot appear in any access pattern.  The
program is written for i0 = 8j - 112 and all c-dependence lives in
the data:
  * rightp input = right pre-shifted by +c columns, zero-padded to
    W+8 columns -- the program's static gather rightp[w - i0] then
    yields right[w - i] with the out-of-range mask applied by the
    padding.
  * left-half validity: inside the union span the mask can differ
    from 1 only on a 7-column edge (upper bound w < W + i0 + c for
    the i0 < 0 planes, lower bound w >= i0 + c for the i0 >= 0
    planes).  The interior is copied raw; the edge gets one tiny
    multiply with a 7-wide 0/1 mask built once from an iota and the
    per-core scalar input cvec (= c).
Each plane writes the union-over-c of valid w-spans; columns inside
the union but outside the core's true span receive exact zeros from
the padding / mask, so every byte of the output buffer is written.
"""

import os
import sys

sys.path.insert(0, "/opt/trn_rl_repo")

import numpy as np

B, H, W, C = 2, 96, 192, 16
D = 128
MIN_DISP = -112
N_CORES = 8
DPC = D // N_CORES         # 16 disparity planes per core
PAD = 8                    # rightp padded to W + PAD source columns
WP = W + PAD
ROWS = B * H

_CACHE = {}


def _plane_span(j):
    """Union-over-c valid w-span for plane j (program-static)."""
    i0 = 8 * j + MIN_DISP
    if i0 < 0:
        us, ue = 0, min(W + i0 + (N_CORES - 1), W)
    else:
        us, ue = i0, W
    return i0, us, ue


def _blocks():
    """Packed flat-output offsets: per plane one [H, B, nw, 2C] block."""
    off = {}
    o = 0
    for j in range(DPC):
        i0, us, ue = _plane_span(j)
        off[j] = o
        o += ROWS * (ue - us) * 2 * C
    return off, o


def _plane_order():
    """Zipper: widest, narrowest, 2nd-widest, ... balances DMA rings."""
    return sorted(range(DPC), key=lambda j: _plane_span(j)[1] - _plane_span(j)[2])


def _build_program():
    from concourse import bacc, mybir
    import concourse.tile as tile

    nc = bacc.Bacc(
        "TRN2", target_bir_lowering=False, debug=False, num_devices=N_CORES
    )
    i8 = mybir.dt.int8
    bf16 = mybir.dt.bfloat16
    f32 = mybir.dt.float32
    # Batch-packed inputs: partition q = image row h, free dims [b, cols].
    left = nc.dram_tensor("left", [H, B * W * C], i8, kind="ExternalInput")
    rightp = nc.dram_tensor("rightp", [H, B * WP * C], i8, kind="ExternalInput")
    cvec = nc.dram_tensor("cvec", [128, 1], f32, kind="ExternalInput")
    offs, total = _blocks()
    out = nc.dram_tensor("out", [total], i8, kind="ExternalOutput")

    def dst(j, nw):
        # Block memory order is [b, h, nw*2C] (b-major) so the host
        # dequant needs no [H,B] transpose; the permuting rearrange
        # presents it [q=h][r=b][x] to match the SBUF-side iteration.
        o = offs[j]
        return out.ap()[o : o + ROWS * nw * 2 * C].rearrange(
            "(r q x) -> q r x", q=H, r=B
        )

    E = 7  # edge width: union span minus guaranteed-valid interior

    with tile.TileContext(nc) as tc:
        with (
            tc.tile_pool(name="inputs", bufs=1) as ipool,
            tc.tile_pool(name="work", bufs=4) as wpool,
        ):
            lsb = ipool.tile([128, B * W * C], i8, tag="lsb")
            rsb = ipool.tile([128, B * WP * C], i8, tag="rsb")
            cv = ipool.tile([128, 1], f32, tag="cvec")
            tmpi = ipool.tile([128, B * E * C], bf16, tag="tmpi")
            emb = ipool.tile([128, B * E * C], bf16, tag="emb")
            em_lt = ipool.tile([128, B * E * C], i8, tag="em_lt")
            em_ge = ipool.tile([128, B * E * C], i8, tag="em_ge")

            # One load per input at the heads of the two (empty) HWDGE
            # store rings; cvec (512 B) leads the scalar ring.  iota
            # runs on GpSimd immediately (no deps).
            nc.scalar.dma_start(cv[:, :], cvec.ap())
            nc.sync.dma_start(rsb[0:96, :], rightp.ap())
            nc.scalar.dma_start(lsb[0:96, :], left.ap())
            nc.gpsimd.iota(
                tmpi[:, :], [[0, B], [1, E], [0, C]], channel_multiplier=0,
                allow_small_or_imprecise_dtypes=True,
            )

            # Edge masks over dw = 0..6 (batch- and channel-expanded),
            # exact integer compares in bf16, cast to int8 once:
            # em_lt[dw] = dw < c  (right edge of i0 < 0 planes),
            # em_ge[dw] = dw >= c (left edge of i0 >= 0 planes).
            nc.vector.tensor_single_scalar(
                emb[:, :], tmpi[:, :], cv[:, 0:1], mybir.AluOpType.is_lt
            )
            nc.scalar.copy(em_lt[:, :], emb[:, :])
            nc.vector.tensor_single_scalar(
                emb[:, :], tmpi[:, :], cv[:, 0:1], mybir.AluOpType.is_ge
            )
            nc.scalar.copy(em_ge[:, :], emb[:, :])

            lv = lsb[0:96, :].rearrange("p (r w y) -> p r w y", r=B, w=W)
            rv = rsb[0:96, :].rearrange("p (r w y) -> p r w y", r=B, w=WP)
            elv = em_lt[0:96, :].rearrange("p (r w y) -> p r w y", r=B, w=E)
            egv = em_ge[0:96, :].rearrange("p (r w y) -> p r w y", r=B, w=E)

            for k, j in enumerate(_plane_order()):
                i0, us, ue = _plane_span(j)
                nw = ue - us
                x0 = us - i0      # source column offset into rightp
                ring = (nc.sync, nc.scalar)[k % 2]
                # Edge start and always-valid interior [n0, n1).
                if i0 < 0:
                    e0, n0, n1, em = ue - E, us, ue - E, elv
                else:
                    e0, n0, n1, em = us, us + E, ue, egv

                cat = wpool.tile([128, B * W * 2 * C], i8, tag="cat")
                cw = cat[0:96, :].rearrange("p (r w y) -> p r w y", r=B, w=W)

                # Left interior (validity provably 1 for every core):
                # raw copy on the vector engine.
                nc.vector.tensor_copy(
                    cw[:, :, n0 - us : n1 - us, 0:C], lv[:, :, n0:n1, :]
                )
                # Left edge (<= 7 columns where the mask may be 0):
                # one tiny multiply.
                nc.vector.tensor_mul(
                    cw[:, :, e0 - us : e0 - us + E, 0:C],
                    lv[:, :, e0 : e0 + E, :],
                    em[:, :, :, :],
                )
                # Shifted right window into the odd channel half
                # (scalar engine; the disparity shift is just a byte
                # offset in the source AP).
                nc.scalar.copy(
                    cw[:, :, 0:nw, C : 2 * C], rv[:, :, x0 : x0 + nw, :]
                )

                # One DMA per plane: [h, b, nw*2C] with a contiguous
                # nw*2C inner run on both sides.
                ring.dma_start(
                    dst(j, nw),
                    cat[0:96, :].rearrange("p (r x) -> p r x", r=B)[
                        :, :, 0 : nw * 2 * C
                    ],
                )

    nc.compile()
    return nc


def _get_program():
    if "nc" not in _CACHE:
        _CACHE["nc"] = _build_program()
    return _CACHE["nc"]


def _get_runner():
    """SPMD executor.

    On a native TRN host (no axon), defer to run_bass_kernel_spmd --
    the NrtSession path it picks there has local DMA and needs no help.

    Under axon (remote-accelerator PJRT proxy, ~30 MB/s tunnel), use
    the multi-core body of bass2jax.run_bass_via_pjrt -- the exact path
    run_bass_kernel_spmd takes under axon -- hoisted out of the
    per-call path so the jitted shard_map is built once, and with the
    donated ExternalOutput buffers created ON DEVICE (a jnp.zeros jit)
    instead of uploading ~114 MB of host zeros through the tunnel on
    every call.  This program writes every output byte, so the donated
    buffer's contents are never observed anyway.
    """
    if "runner" in _CACHE:
        return _CACHE["runner"]

    from concourse._compat import axon_active

    if not axon_active():
        from concourse.bass_utils import run_bass_kernel_spmd

        nc = _get_program()

        def run_native(in_maps):
            res = run_bass_kernel_spmd(
                nc, in_maps, core_ids=list(range(N_CORES))
            )
            return [res.results[c]["out"] for c in range(N_CORES)]

        _CACHE["runner"] = run_native
        return run_native

    import jax
    import jax.numpy as jnp
    from jax.sharding import Mesh, NamedSharding, PartitionSpec
    from jax.experimental.shard_map import shard_map
    from concourse import bass2jax, mybir

    nc = _get_program()
    bass2jax.install_neuronx_cc_hook()

    partition_name = (
        nc.partition_id_tensor.name if nc.partition_id_tensor else None
    )
    in_names, out_names, out_avals = [], [], []
    for alloc in nc.m.functions[0].allocations:
        if not isinstance(alloc, mybir.MemoryLocationSet):
            continue
        name = alloc.memorylocations[0].name
        if alloc.kind == "ExternalInput":
            if name != partition_name:
                in_names.append(name)
        elif alloc.kind == "ExternalOutput":
            out_names.append(name)
            out_avals.append(
                jax.core.ShapedArray(
                    tuple(alloc.tensor_shape), mybir.dt.np(alloc.dtype)
                )
            )
    n_params = len(in_names)
    all_names = list(in_names) + out_names
    if partition_name is not None:
        all_names.append(partition_name)

    def _body(*args):
        operands = list(args)
        if partition_name is not None:
            operands.append(bass2jax.partition_id_tensor())
        outs = bass2jax._bass_exec_p.bind(
            *operands,
            out_avals=tuple(out_avals),
            in_names=tuple(all_names),
            out_names=tuple(out_names),
            lowering_input_output_aliases=(),
            sim_require_finite=True,
            sim_require_nnan=True,
            nc=nc,
        )
        return tuple(outs)

    devices = jax.devices()[:N_CORES]
    mesh = Mesh(np.asarray(devices), ("core",))
    donate = tuple(range(n_params, n_params + len(out_names)))
    sharded = jax.jit(
        shard_map(
            _body,
            mesh=mesh,
            in_specs=(PartitionSpec("core"),) * (n_params + len(out_names)),
            out_specs=(PartitionSpec("core"),) * len(out_names),
            check_rep=False,
        ),
        donate_argnums=donate,
        keep_unused=True,
    )
    zshape = (N_CORES * out_avals[0].shape[0], *out_avals[0].shape[1:])
    zeros_fn = jax.jit(
        lambda: jnp.zeros(zshape, jnp.int8),
        out_shardings=NamedSharding(mesh, PartitionSpec("core")),
    )

    def run(in_maps):
        """Returns the sharded jax output array; the caller fetches
        its shards (kernel() overlaps the fetch with dequantization)."""
        concat_in = [
            np.concatenate([m[name] for m in in_maps], axis=0)
            for name in in_names
        ]
        return sharded(*concat_in, zeros_fn())[0]

    _CACHE["sharded"] = sharded
    _CACHE["zeros_fn"] = zeros_fn
    _CACHE["in_names"] = in_names
    _CACHE["runner"] = run
    return run


def kernel(left, right):
    left = np.ascontiguousarray(left, dtype=np.float32)
    right = np.ascontiguousarray(right, dtype=np.float32)
    scale = max(np.abs(left).max(), np.abs(right).max()) / 127.0
    scale = float(scale) if scale > 0 else 1.0
    left_q = np.clip(np.rint(left / scale), -127, 127).astype(np.int8)
    right_q = np.clip(np.rint(right / scale), -127, 127).astype(np.int8)
    # Batch-packed: [h, b, cols].
    left_t = np.ascontiguousarray(
        left_q.reshape(B, H, W * C).transpose(1, 0, 2)
    ).reshape(H, B * W * C)

    in_maps = []
    for c in range(N_CORES):
        rp = np.zeros((B, H, WP, C), dtype=np.int8)
        rp[:, :, c : c + W] = right_q
        rp_t = np.ascontiguousarray(
            rp.reshape(B, H, WP * C).transpose(1, 0, 2)
        ).reshape(H, B * WP * C)
        cvv = np.full((128, 1), float(c), dtype=np.float32)
        in_maps.append({"left": left_t, "rightp": rp_t, "cvec": cvv})

    run = _get_runner()
    prof_dir = os.environ.get("BASS_NTFF_DIR")
    ctx = None
    if prof_dir:
        try:  # optional NTFF capture; absent outside the axon dev env
            from trn_agent_boot.trn_boot import _ntff_profile_via_ctypes

            ctx = _ntff_profile_via_ctypes("/opt/axon/libaxon_pjrt.so")(
                prof_dir, [0]
            )
        except Exception:
            ctx = None
    if ctx is None:
        import contextlib

        ctx = contextlib.nullcontext()

    # Gather: one dequantizing multiply per plane-block into the final
    # [B, D, H, W, 2C] array (d = 8j + c).  Blocks arrive [b, h, nw, 2C]
    # so no transpose is needed; invalid columns outside the union
    # spans stay zero from calloc.
    offs, total = _blocks()
    s32 = np.float32(scale)
    full = np.zeros((B, DPC, N_CORES, H, W, 2 * C), dtype=np.float32)

    def unpack_core(c, flat):
        for j in range(DPC):
            i0, us, ue = _plane_span(j)
            nw = ue - us
            o = offs[j]
            blk = flat[o : o + ROWS * nw * 2 * C].reshape(B, H, nw, 2 * C)
            np.multiply(blk, s32, out=full[:, j, c, :, us:ue, :])

    with ctx:
        results = run(in_maps)
        if hasattr(results, "addressable_shards"):
            # Axon path: fetch the 8 shards on concurrent threads (the
            # D2H transport runs in C with the GIL released) and
            # dequantize each core's blocks as it lands, overlapping
            # host math with the remaining wire time.
            import queue
            import threading

            q = queue.Queue()

            def fetch(sh):
                try:
                    q.put((sh.index[0].start // total, np.asarray(sh.data)))
                except BaseException as e:
                    q.put(e)

            threads = [
                threading.Thread(target=fetch, args=(sh,))
                for sh in results.addressable_shards
            ]
            for t in threads:
                t.start()
            for _ in range(N_CORES):
                item = q.get()
                if isinstance(item, BaseException):
                    for t in threads:
                        t.join()
                    raise item
                unpack_core(*item)
            for t in threads:
                t.join()
        else:
            # Native path: run_bass_kernel_spmd already returned
            # host-resident per-core arrays.
            for c in range(N_CORES):
                unpack_core(c, results[c])
    return full.reshape(B, D, H, W, 2 * C)
